# revision 6
# baseline (speedup 1.0000x reference)
# kernel.py — AgentAttention on 8 Trainium2 NeuronCores (self-contained).
#
# Problem (per batch b, head h):
#   qq  = softmax(q @ a, axis=-1)            # [N, d] over agents d
#   kk  = softmax(a @ k, axis=-1)            # [d, N] over keys N
#   out = qq @ (kk @ v)                      # [N, d]
# Shapes: q [8,16,2048,128], a [8,16,128,128], k [8,16,128,2048],
#         v [8,16,2048,128]; d == n_agents == 128.
#
# Sharding: batch dimension (8) across the 8 cores; each core computes its
# 16 heads independently (pure data parallel, no collectives).
#
# The kernel is HBM-bound (~34MB of fp16/bf16 I/O per core at ~358 GB/s),
# so the structure is built around keeping the DMA rings saturated:
#   - inputs are packed into TWO per-head DRAM blobs so each head costs two
#     big contiguous loads (8KB resp. 4.6KB per partition line): qk blob on
#     the sync HWDGE ring, v+a+aT blob on the scalar HWDGE ring, prefetched
#     two heads ahead (nothing on those rings can ever stall).
#   - the whole head's output is accumulated in SBUF and stored with ONE
#     512KB DMA (4KB lines) per head on the gpsimd SWDGE ring, so the
#     store's data-ready wait can never block load descriptor generation.
# Per-head device algorithm (all matmuls contract over the partition dim):
#   s2T[m,j] = (a@k)^T via lhsT=kp-chunk (host-prepermuted, contiguous),
#              rhs=aT                                              (fp16)
#   e2   = exp(s2T) -> bf16    (no max subtraction: |logit| <~ 70 < 88.7)
#   agg|S = sum_m e2[m,:]^T @ [v_m | 1]  (bf16 matmuls into fp32 psum);
#           col 128 is S_j = sum_m exp, i.e. the kk softmax denominator
#   aggN = agg / S_j                           (row scale, bf16)
#   s1T[j,n] = (q@a)^T via lhsT=a, rhs=qT (pre-transposed)         (fp16)
#   e1   = exp(s1T) -> bf16
#   out  = (e1-chunk^T @ [aggN | 1]) / T_n; the aggN ones column makes
#          col 128 of each psum chunk T_n (the qq softmax denominator)
# fp16 for the logit matmuls: 11-bit mantissa (tf32-class accuracy,
# ~5e-3 abs-max rel err vs 2e-2 budget). The fp16 output is upcast to
# fp32 on the host; q/k/v row permutations are also host-side.

import numpy as np

B, H, N, D = 8, 16, 2048, 128
NCORES = 8
NCH = N // D  # 16 chunks of 128 along the sequence dim

CONFIG = {
    "trace": False,
}

_PROGRAM_CACHE = {}

# per-partition fp16-element offsets into the vab blob
V_ELEMS = NCH * (D + 1)          # 2064 bf16 values (v rows + ones cols)
A_OFF = V_ELEMS                  # a at [2064, 2192)
AT_OFF = V_ELEMS + D             # aT at [2192, 2320)
VAB_W = V_ELEMS + 2 * D          # 2320 fp16 elems = 4640B per partition


def _patch_tile_drain():
    """This container's walrus rejects >1 sync-wait on a Drain instruction
    (CoreV3GenImpl setupSyncWait). Split the TileContext tail-drain's waits
    across consecutive single-wait drains on the same engine; semantics are
    identical (program order ANDs the waits)."""
    import concourse.tile as tile_mod
    from concourse import mybir
    from concourse.tile import ScopedClock

    if getattr(tile_mod.TileContext, "_agentattn_drain_patched", False):
        return

    def _drain_and_barrier(self, tick_clock, wait_clock):
        nc = self.nc
        drain_inst = nc.sync.drain()
        wait_clock.add_sem_waits(
            drain_inst.ins, ScopedClock({None: tick_clock.global_clock})
        )
        si = drain_inst.ins.sync_info
        if si is not None and si.on_wait and len(si.on_wait) > 1:
            waits = list(si.on_wait)
            ups = list(si.on_update or [])
            drain_inst.ins.sync_info = mybir.SyncInfo(
                on_wait=waits[:1], on_update=ups
            )
            for w in waits[1:]:
                d2 = nc.sync.drain()
                d2.ins.sync_info = mybir.SyncInfo(on_wait=[w], on_update=[])
        nc.all_engine_barrier()
        assert self.sems is not None
        popped = nc._tile_sem_poison_stack.pop()
        assert popped is self._sem_poison
        nc.clear_and_free_semaphores(list(self.sems.allocated().values()))
        nc.all_engine_barrier()

    tile_mod.TileContext._drain_and_barrier = _drain_and_barrier
    tile_mod.TileContext._agentattn_drain_patched = True


def _split_sync_waits(nc, max_waits=1):
    """This container's walrus rejects instructions carrying more than one
    sync-wait command. Hoist excess waits onto same-engine NOPs inserted
    immediately before the instruction (program order on the engine ANDs
    the waits, so semantics are unchanged)."""
    from concourse import mybir

    n_split = 0
    for fn in nc.m.functions:
        for blk in fn.blocks:
            insts = blk.instructions
            if not any(
                (si := inst.sync_info) is not None
                and si.on_wait
                and len(si.on_wait) > max_waits
                for inst in insts
            ):
                continue
            new = []
            for inst in insts:
                si = inst.sync_info
                if si is not None and si.on_wait and len(si.on_wait) > max_waits:
                    waits = list(si.on_wait)
                    for idx, w in enumerate(waits[:-max_waits]):
                        nop = mybir.InstNoOp(
                            name=f"{inst.name}_hw{idx}", ins=[], outs=[]
                        )
                        nop.engine = inst.engine
                        nop.sync_info = mybir.SyncInfo(on_wait=[w], on_update=[])
                        nc.register_instruction(nop)
                        new.append(nop)
                        n_split += 1
                    inst.sync_info = mybir.SyncInfo(
                        on_wait=waits[-max_waits:],
                        on_update=list(si.on_update or []),
                    )
                new.append(inst)
            blk.instructions = new
    return n_split


def install_ntff_hook():
    """Make trace=True work in this container: provide the antenv.axon_hooks
    shim that run_bass_kernel_spmd expects, backed by the injected
    libaxon_pjrt.so, and stub out the artifact upload."""
    import sys, types
    if "antenv.axon_hooks" not in sys.modules:
        from trn_agent_boot.trn_boot import _ntff_profile_via_ctypes
        hook = _ntff_profile_via_ctypes("/opt/axon/libaxon_pjrt.so")
        mod = types.ModuleType("antenv.axon_hooks")
        mod.get_axon_ntff_profile_hook = lambda: hook
        mod.set_axon_ntff_profile_hook = lambda h: None
        sys.modules["antenv.axon_hooks"] = mod
    import concourse.bass_utils as bu
    bu.upload_artifacts = lambda tmpdir: tmpdir


def build_program(cfg=None):
    """Build the single-core Bass program (16 heads of agent attention).

    Host-baked DRAM layout (see kernel()):
      qk  [H, 128, 4096] fp16: per partition i: qT[i, 0:2048] | kp[i, 0:2048]
          where qT[d, n'] = q[n, d] with n-position n' = c*128 + p for query
          row n = p*16 + c (the output-store row permutation), and
          kp[i, mi*128 + ml] = k[i, 16*ml + mi] (chunk-contiguous so the s2
          stationary operand loads from a contiguous slice).
      vab [H, 128, 2320] fp16-sized: per partition p:
          v-perm bf16 [16 chunks x 129] (row n = p*16 + c, a 1.0 column
          after every 128 values: feeds the kk softmax denominator) |
          a fp16 [128] | aT fp16 [128]
      o   [H, 128, 16, 128] fp16: partition p, chunk c = output row p*16+c
          (flattening (p,c) IS the natural row order).
    """
    import concourse.bass as bass
    import concourse.tile as tile
    from concourse import mybir
    from contextlib import ExitStack

    _patch_tile_drain()

    f32 = mybir.dt.float32
    f16 = mybir.dt.float16
    bf16 = mybir.dt.bfloat16
    EXP = mybir.ActivationFunctionType.Exp
    MUL = mybir.AluOpType.mult

    nc = bass.Bass("TRN2", target_bir_lowering=False, debug=False)
    qk_d = nc.dram_tensor("qk", [H, 128, 2 * N], f16, kind="ExternalInput").ap()
    vab_d = nc.dram_tensor("vab", [H, 128, VAB_W], f16, kind="ExternalInput").ap()
    o_d = nc.dram_tensor("o", [H, 128, NCH, D], f16, kind="ExternalOutput").ap()

    with tile.TileContext(nc) as tc, ExitStack() as ctx:
        p_qk = ctx.enter_context(tc.tile_pool(name="p_qk", bufs=4))
        p_vab = ctx.enter_context(tc.tile_pool(name="p_vab", bufs=4))
        p_e2 = ctx.enter_context(tc.tile_pool(name="p_e2", bufs=3))
        p_e1 = ctx.enter_context(tc.tile_pool(name="p_e1", bufs=3))
        p_o = ctx.enter_context(tc.tile_pool(name="p_o", bufs=3))
        p_sm = ctx.enter_context(tc.tile_pool(name="p_sm", bufs=3))

        # 8 psum banks exactly: 2x 2-bank work tiles + 1 agg + 3 out
        ps_work = ctx.enter_context(tc.tile_pool(name="ps_work", bufs=2, space="PSUM"))
        ps_aggp = ctx.enter_context(tc.tile_pool(name="ps_agg", bufs=1, space="PSUM"))
        ps_out = ctx.enter_context(tc.tile_pool(name="ps_out", bufs=3, space="PSUM"))

        def load_head(h):
            # two big contiguous loads per head on separate HWDGE rings;
            # nothing else runs on those rings ahead of them, so descriptor
            # generation can never be blocked by a data-dependent wait.
            qk_sb = p_qk.tile([128, 2 * N], f16, tag="qk")
            nc.sync.dma_start(qk_sb, qk_d[h])
            vab_sb = p_vab.tile([128, VAB_W], f16, tag="vab")
            nc.scalar.dma_start(vab_sb, vab_d[h])
            return qk_sb, vab_sb

        pending = [load_head(0), load_head(1)]

        def emit_out(h, e1_sb, aggN):
            # out[n, v] = (sum_j e1[j, n] aggN[j, v]) / T_n; the ones column
            # of aggN makes column 128 of each product chunk equal T_n.
            # Three 129-wide chunks share one psum bank; one strided
            # reciprocal covers the bank's three T columns. The whole head
            # accumulates in o_sb and goes out as ONE 512KB store.
            o_sb = p_o.tile([128, NCH, D], f16, tag="o_sb")
            GRP = [(0, 3), (3, 3), (6, 3), (9, 3), (12, 3), (15, 1)]
            for g0, gn in GRP:
                pso = ps_out.tile([128, 512], f32, tag="out")
                for i in range(gn):
                    ni = g0 + i
                    nc.tensor.matmul(
                        pso[:, i * (D + 1) : (i + 1) * (D + 1)],
                        lhsT=e1_sb[:, ni * D : (ni + 1) * D], rhs=aggN,
                        start=True, stop=True,
                    )
                rcT = p_sm.tile([128, 3], f32, tag="rcT")
                nc.vector.reciprocal(rcT[:, :gn], pso[:, D :: D + 1][:, :gn])
                # one 3D multiply per psum bank: o[:, g, :] = pso[:, g, 0:D]
                # * rcT[:, g] (inner dim broadcast via stride-0)
                nc.vector.tensor_tensor(
                    o_sb[:, g0 : g0 + gn, :],
                    pso[:, 0 : gn * (D + 1)].rearrange(
                        "p (g c) -> p g c", c=D + 1
                    )[:, :, 0:D],
                    rcT[:, 0:gn].to_broadcast((128, gn, D)),
                    MUL,
                )
            # single whole-head store on the SWDGE ring: its data-ready wait
            # only ever blocks the (idle) gpsimd engine.
            nc.gpsimd.dma_start(o_d[h], o_sb)

        carry = None  # (e1_sb, aggN) of the previous head
        for h in range(H):
            qk_sb, vab_sb = pending[0]
            qT_sb = qk_sb[:, 0:N]
            kp_sb = qk_sb[:, N : 2 * N]
            a_sb = vab_sb[:, A_OFF : A_OFF + D]
            aT_sb = vab_sb[:, AT_OFF : AT_OFF + D]
            v_bf = vab_sb[:, 0:V_ELEMS].bitcast(bf16)

            # s2T[ml, j] for key m = 16*ml + mi; two 2-bank psum halves, one
            # 1024-wide exp each (amortizes the ~270ns ACT fixed cost).
            e2_sb = p_e2.tile([128, N], bf16, tag="e2")
            for half in range(2):
                ps = ps_work.tile([128, 1024], f32, tag="work")
                for t in range(8):
                    mi = half * 8 + t
                    nc.tensor.matmul(
                        ps[:, t * D : (t + 1) * D],
                        lhsT=kp_sb[:, mi * D : (mi + 1) * D], rhs=aT_sb,
                        start=True, stop=True,
                    )
                nc.scalar.activation(
                    e2_sb[:, half * 1024 : (half + 1) * 1024], ps, EXP
                )

            # previous head's output stage slots into the PE's wait for the
            # e2 exps (software pipelining by one head); by the time it's
            # done the first work tile is free again for s1.
            if carry is not None:
                emit_out(h - 1, *carry)

            # s1T[j, n] = sum_i a[i, j] qT[i, n] — BEFORE agg so the e1 exps
            # follow the e2 exps back-to-back on the scalar engine (the two
            # exp chains per head set the pipeline period).
            e1_sb = p_e1.tile([128, N], bf16, tag="e1")
            for half in range(2):
                ps = ps_work.tile([128, 1024], f32, tag="work")
                for t in range(2):
                    c = half * 2 + t
                    nc.tensor.matmul(
                        ps[:, t * 512 : (t + 1) * 512],
                        lhsT=a_sb, rhs=qT_sb[:, c * 512 : (c + 1) * 512],
                        start=True, stop=True,
                    )
                nc.scalar.activation(
                    e1_sb[:, half * 1024 : (half + 1) * 1024], ps, EXP
                )

            # agg[j, 0:128] = sum_m e2[m, j] v[m, :];  agg[j, 128] = S_j
            # (via the ones column baked into the v blob on the host)
            agg = ps_aggp.tile([128, D + 1], f32, tag="agg")
            for mi in range(NCH):
                nc.tensor.matmul(
                    agg,
                    lhsT=e2_sb[:, mi * D : (mi + 1) * D],
                    rhs=v_bf[:, mi * (D + 1) : (mi + 1) * (D + 1)],
                    start=(mi == 0), stop=(mi == NCH - 1),
                )
            recipS = p_sm.tile([128, 1], f32, tag="recipS")
            nc.vector.reciprocal(recipS, agg[:, D : D + 1])
            # aggN has a trailing ones column: the output matmul then yields
            # T_n (the qq softmax denominator) in its own column 128.
            aggN = p_sm.tile([128, D + 1], bf16, tag="aggN")
            nc.gpsimd.memset(aggN[:, D : D + 1], 1.0)
            nc.vector.tensor_tensor(
                aggN[:, 0:D], agg[:, 0:D], recipS.to_broadcast((128, D)), MUL
            )

            # prefetch two heads ahead (bufs=4 keeps the pool wait clear of
            # any head still in flight)
            if h + 2 < H:
                pending.append(load_head(h + 2))
            pending.pop(0)

            carry = (e1_sb, aggN)

        emit_out(H - 1, *carry)

    _split_sync_waits(nc)
    return nc


def _get_program(cfg_key):
    if cfg_key not in _PROGRAM_CACHE:
        _PROGRAM_CACHE[cfg_key] = build_program()
    return _PROGRAM_CACHE[cfg_key]


def kernel(q, a, k, v):
    from concourse.bass_utils import run_bass_kernel_spmd

    import ml_dtypes

    # Device I/O is 16-bit and pre-laid-out: the kernel contracts logits in
    # fp16 and values in bf16 anyway, so rounding + transposing on the host
    # halves HBM traffic and removes all on-chip transposes at no extra
    # precision cost. The fp16 output is upcast back to fp32 below.
    assert np.asarray(q).shape == (B, H, N, D)
    # qT's free dim uses the same row permutation as the output store
    # (position c*128 + p holds query row n = p*16 + c)
    qt = (
        np.asarray(q, dtype=np.float16)
        .reshape(B, H, 128, NCH, D)
        .transpose(0, 1, 4, 3, 2)
        .reshape(B, H, D, N)
    )
    # k chunk-permuted: kp[i, mi*128 + ml] = k[i, 16*ml + mi] so each s2
    # stationary operand is a contiguous 128-col slice
    kp = (
        np.asarray(k, dtype=np.float16)
        .reshape(B, H, D, D, NCH)
        .transpose(0, 1, 2, 4, 3)
        .reshape(B, H, D, N)
    )
    qk = np.empty((B, H, 128, 2 * N), dtype=np.float16)
    qk[..., 0:N] = qt
    qk[..., N : 2 * N] = kp
    qk = np.ascontiguousarray(qk)

    a16 = np.asarray(a, dtype=np.float16)
    # v: rows permuted to n = p*16 + c, a 1.0 column after every 128 values
    # (feeds the kk softmax denominator out of the agg matmul)
    vab = np.empty((B, H, 128, VAB_W), dtype=np.uint16)
    vp = np.ones((B, H, 128, NCH, D + 1), dtype=ml_dtypes.bfloat16)
    vp[..., 0:D] = np.asarray(v, dtype=ml_dtypes.bfloat16).reshape(
        B, H, 128, NCH, D
    )
    vab[..., 0:V_ELEMS] = vp.view(np.uint16).reshape(B, H, 128, V_ELEMS)
    # a_sb[i, j] = a[i, j] (s1 stationary); aT_sb[i, j] = a[j, i] (s2 moving)
    vab[..., A_OFF : A_OFF + D] = a16.view(np.uint16)
    vab[..., AT_OFF : AT_OFF + D] = np.ascontiguousarray(
        a16.transpose(0, 1, 3, 2)
    ).view(np.uint16)
    vab = np.ascontiguousarray(vab).view(np.float16)

    nc = _get_program(("main",))
    core_ids = list(range(NCORES))
    in_maps = [{"qk": qk[c], "vab": vab[c]} for c in core_ids]
    res = run_bass_kernel_spmd(nc, in_maps, core_ids, trace=CONFIG["trace"])
    # o is [H, 128, 16, 128] with (p, c) flattening = natural row n = p*16+c
    out = np.stack(
        [
            np.asarray(res.results[c]["o"], dtype=np.float32).reshape(H, N, D)
            for c in core_ids
        ]
    )
    kernel.last_result = res
    return out


# revision 9
# speedup vs baseline: 1.1159x; 1.1159x over previous
# kernel.py — AgentAttention on 8 Trainium2 NeuronCores (self-contained).
#
# Problem (per batch b, head h):
#   qq  = softmax(q @ a, axis=-1)            # [N, d] over agents d
#   kk  = softmax(a @ k, axis=-1)            # [d, N] over keys N
#   out = qq @ (kk @ v)                      # [N, d]
# Shapes: q [8,16,2048,128], a [8,16,128,128], k [8,16,128,2048],
#         v [8,16,2048,128]; d == n_agents == 128.
#
# Sharding: batch dimension (8) across the 8 cores; each core computes its
# 16 heads independently (pure data parallel, no collectives).
#
# The kernel is HBM-bound (~34MB of fp16/bf16 I/O per core at ~358 GB/s),
# so the structure is built around keeping the DMA rings saturated:
#   - inputs are packed into TWO per-head DRAM blobs so each head costs two
#     big contiguous loads (8KB resp. 4.6KB per partition line): qk blob on
#     the sync HWDGE ring, v+a+aT blob on the scalar HWDGE ring, prefetched
#     two heads ahead (nothing on those rings can ever stall).
#   - the whole head's output is accumulated in SBUF and stored with ONE
#     512KB DMA (4KB lines) per head on the gpsimd SWDGE ring, so the
#     store's data-ready wait can never block load descriptor generation.
# Per-head device algorithm (all matmuls contract over the partition dim):
#   s2T[m,j] = (a@k)^T via lhsT=kp-chunk (host-prepermuted, contiguous),
#              rhs=aT                                              (fp16)
#   e2   = exp(s2T) -> bf16    (no max subtraction: |logit| <~ 70 < 88.7)
#   agg|S = sum_m e2[m,:]^T @ [v_m | 1]  (bf16 matmuls into fp32 psum);
#           col 128 is S_j = sum_m exp, i.e. the kk softmax denominator
#   aggN = agg / S_j                           (row scale, bf16)
#   s1T[j,n] = (q@a)^T via lhsT=a, rhs=qT (pre-transposed)         (fp16)
#   e1   = exp(s1T) -> bf16
#   out  = (e1-chunk^T @ [aggN | 1]) / T_n; the aggN ones column makes
#          col 128 of each psum chunk T_n (the qq softmax denominator)
# fp16 for the logit matmuls: 11-bit mantissa (tf32-class accuracy,
# ~5e-3 abs-max rel err vs 2e-2 budget). The fp16 output is upcast to
# fp32 on the host; q/k/v row permutations are also host-side.

import numpy as np

B, H, N, D = 8, 16, 2048, 128
NCORES = 8
NCH = N // D  # 16 chunks of 128 along the sequence dim

CONFIG = {
    "trace": False,
}

_PROGRAM_CACHE = {}

# per-partition fp16-element offsets into the vab blob
V_ELEMS = NCH * (D + 1)          # 2064 bf16 values (v rows + ones cols)
A_OFF = V_ELEMS                  # a at [2064, 2192)
AT_OFF = V_ELEMS + D             # aT at [2192, 2320)
VAB_W = V_ELEMS + 2 * D          # 2320 fp16 elems = 4640B per partition


def _patch_tile_drain():
    """This container's walrus rejects >1 sync-wait on a Drain instruction
    (CoreV3GenImpl setupSyncWait). Split the TileContext tail-drain's waits
    across consecutive single-wait drains on the same engine; semantics are
    identical (program order ANDs the waits)."""
    import concourse.tile as tile_mod
    from concourse import mybir
    from concourse.tile import ScopedClock

    if getattr(tile_mod.TileContext, "_agentattn_drain_patched", False):
        return

    def _drain_and_barrier(self, tick_clock, wait_clock):
        nc = self.nc
        drain_inst = nc.sync.drain()
        wait_clock.add_sem_waits(
            drain_inst.ins, ScopedClock({None: tick_clock.global_clock})
        )
        si = drain_inst.ins.sync_info
        if si is not None and si.on_wait and len(si.on_wait) > 1:
            waits = list(si.on_wait)
            ups = list(si.on_update or [])
            drain_inst.ins.sync_info = mybir.SyncInfo(
                on_wait=waits[:1], on_update=ups
            )
            for w in waits[1:]:
                d2 = nc.sync.drain()
                d2.ins.sync_info = mybir.SyncInfo(on_wait=[w], on_update=[])
        nc.all_engine_barrier()
        assert self.sems is not None
        popped = nc._tile_sem_poison_stack.pop()
        assert popped is self._sem_poison
        nc.clear_and_free_semaphores(list(self.sems.allocated().values()))
        nc.all_engine_barrier()

    tile_mod.TileContext._drain_and_barrier = _drain_and_barrier
    tile_mod.TileContext._agentattn_drain_patched = True


def _split_sync_waits(nc, max_waits=1):
    """This container's walrus rejects instructions carrying more than one
    sync-wait command. Hoist excess waits onto same-engine NOPs inserted
    immediately before the instruction (program order on the engine ANDs
    the waits, so semantics are unchanged)."""
    from concourse import mybir

    n_split = 0
    for fn in nc.m.functions:
        for blk in fn.blocks:
            insts = blk.instructions
            if not any(
                (si := inst.sync_info) is not None
                and si.on_wait
                and len(si.on_wait) > max_waits
                for inst in insts
            ):
                continue
            new = []
            for inst in insts:
                si = inst.sync_info
                if si is not None and si.on_wait and len(si.on_wait) > max_waits:
                    waits = list(si.on_wait)
                    for idx, w in enumerate(waits[:-max_waits]):
                        nop = mybir.InstNoOp(
                            name=f"{inst.name}_hw{idx}", ins=[], outs=[]
                        )
                        nop.engine = inst.engine
                        nop.sync_info = mybir.SyncInfo(on_wait=[w], on_update=[])
                        nc.register_instruction(nop)
                        new.append(nop)
                        n_split += 1
                    inst.sync_info = mybir.SyncInfo(
                        on_wait=waits[-max_waits:],
                        on_update=list(si.on_update or []),
                    )
                new.append(inst)
            blk.instructions = new
    return n_split


def install_ntff_hook():
    """Make trace=True work in this container: provide the antenv.axon_hooks
    shim that run_bass_kernel_spmd expects, backed by the injected
    libaxon_pjrt.so, and stub out the artifact upload."""
    import sys, types
    if "antenv.axon_hooks" not in sys.modules:
        from trn_agent_boot.trn_boot import _ntff_profile_via_ctypes
        hook = _ntff_profile_via_ctypes("/opt/axon/libaxon_pjrt.so")
        mod = types.ModuleType("antenv.axon_hooks")
        mod.get_axon_ntff_profile_hook = lambda: hook
        mod.set_axon_ntff_profile_hook = lambda h: None
        sys.modules["antenv.axon_hooks"] = mod
    import concourse.bass_utils as bu
    bu.upload_artifacts = lambda tmpdir: tmpdir


def build_program(cfg=None):
    """Build the single-core Bass program (16 heads of agent attention).

    Host-baked DRAM layout (see kernel()):
      qk  [H, 128, 4096] fp16: per partition i: qT[i, 0:2048] | kp[i, 0:2048]
          where qT[d, n'] = q[n, d] with n-position n' = c*128 + p for query
          row n = p*16 + c (the output-store row permutation), and
          kp[i, mi*128 + ml] = k[i, 16*ml + mi] (chunk-contiguous so the s2
          stationary operand loads from a contiguous slice).
      vab [H, 128, 2320] fp16-sized: per partition p:
          v-perm bf16 [16 chunks x 129] (row n = p*16 + c, a 1.0 column
          after every 128 values: feeds the kk softmax denominator) |
          a fp16 [128] | aT fp16 [128]
      o   [H, 128, 16, 128] fp16: partition p, chunk c = output row p*16+c
          (flattening (p,c) IS the natural row order).
    """
    import concourse.bass as bass
    import concourse.tile as tile
    from concourse import mybir
    from contextlib import ExitStack

    _patch_tile_drain()

    f32 = mybir.dt.float32
    f16 = mybir.dt.float16
    bf16 = mybir.dt.bfloat16
    EXP = mybir.ActivationFunctionType.Exp
    MUL = mybir.AluOpType.mult

    nc = bass.Bass("TRN2", target_bir_lowering=False, debug=False)
    qk_d = nc.dram_tensor("qk", [H, 128, 2 * N], f16, kind="ExternalInput").ap()
    vab_d = nc.dram_tensor("vab", [H, 128, VAB_W], f16, kind="ExternalInput").ap()
    o_d = nc.dram_tensor("o", [H, 128, NCH, D], f16, kind="ExternalOutput").ap()

    with tile.TileContext(nc) as tc, ExitStack() as ctx:
        p_qk = ctx.enter_context(tc.tile_pool(name="p_qk", bufs=6))
        p_vab = ctx.enter_context(tc.tile_pool(name="p_vab", bufs=6))
        p_e2 = ctx.enter_context(tc.tile_pool(name="p_e2", bufs=3))
        p_e1 = ctx.enter_context(tc.tile_pool(name="p_e1", bufs=3))
        p_o = ctx.enter_context(tc.tile_pool(name="p_o", bufs=3))
        p_sm = ctx.enter_context(tc.tile_pool(name="p_sm", bufs=3))

        # 8 psum banks exactly: 2x 2-bank work tiles + 1 agg + 3 out
        ps_work = ctx.enter_context(tc.tile_pool(name="ps_work", bufs=2, space="PSUM"))
        ps_aggp = ctx.enter_context(tc.tile_pool(name="ps_agg", bufs=1, space="PSUM"))
        ps_out = ctx.enter_context(tc.tile_pool(name="ps_out", bufs=3, space="PSUM"))

        def load_head(h):
            # two big contiguous loads per head on separate HWDGE rings;
            # nothing else runs on those rings ahead of them, so descriptor
            # generation can never be blocked by a data-dependent wait.
            qk_sb = p_qk.tile([128, 2 * N], f16, tag="qk")
            nc.sync.dma_start(qk_sb, qk_d[h])
            vab_sb = p_vab.tile([128, VAB_W], f16, tag="vab")
            nc.scalar.dma_start(vab_sb, vab_d[h])
            return qk_sb, vab_sb

        PF = 4  # prefetch depth in heads (bufs=6 keeps slot waits clear)
        pending = [load_head(h) for h in range(PF)]

        def emit_out(h, e1_sb, aggN):
            # out[n, v] = (sum_j e1[j, n] aggN[j, v]) / T_n; the ones column
            # of aggN makes column 128 of each product chunk equal T_n.
            # Three 129-wide chunks share one psum bank; one strided
            # reciprocal covers the bank's three T columns. The whole head
            # accumulates in o_sb and goes out as ONE 512KB store.
            o_sb = p_o.tile([128, NCH, D], f16, tag="o_sb")
            GRP = [(0, 3), (3, 3), (6, 3), (9, 3), (12, 3), (15, 1)]
            for g0, gn in GRP:
                pso = ps_out.tile([128, 512], f32, tag="out")
                for i in range(gn):
                    ni = g0 + i
                    nc.tensor.matmul(
                        pso[:, i * (D + 1) : (i + 1) * (D + 1)],
                        lhsT=e1_sb[:, ni * D : (ni + 1) * D], rhs=aggN,
                        start=True, stop=True,
                    )
                rcT = p_sm.tile([128, 3], f32, tag="rcT")
                nc.vector.reciprocal(rcT[:, :gn], pso[:, D :: D + 1][:, :gn])
                # one 3D multiply per psum bank: o[:, g, :] = pso[:, g, 0:D]
                # * rcT[:, g] (inner dim broadcast via stride-0)
                nc.vector.tensor_tensor(
                    o_sb[:, g0 : g0 + gn, :],
                    pso[:, 0 : gn * (D + 1)].rearrange(
                        "p (g c) -> p g c", c=D + 1
                    )[:, :, 0:D],
                    rcT[:, 0:gn].to_broadcast((128, gn, D)),
                    MUL,
                )
            # single whole-head store on the SWDGE ring: its data-ready wait
            # only ever blocks the (idle) gpsimd engine.
            nc.gpsimd.dma_start(o_d[h], o_sb)

        carry = None  # (e1_sb, aggN) of the previous head
        for h in range(H):
            qk_sb, vab_sb = pending[0]
            qT_sb = qk_sb[:, 0:N]
            kp_sb = qk_sb[:, N : 2 * N]
            a_sb = vab_sb[:, A_OFF : A_OFF + D]
            aT_sb = vab_sb[:, AT_OFF : AT_OFF + D]
            v_bf = vab_sb[:, 0:V_ELEMS].bitcast(bf16)

            # s2T[ml, j] for key m = 16*ml + mi; two 2-bank psum halves, one
            # 1024-wide exp each (amortizes the ~270ns ACT fixed cost).
            e2_sb = p_e2.tile([128, N], bf16, tag="e2")
            for half in range(2):
                ps = ps_work.tile([128, 1024], f32, tag="work")
                for t in range(8):
                    mi = half * 8 + t
                    nc.tensor.matmul(
                        ps[:, t * D : (t + 1) * D],
                        lhsT=kp_sb[:, mi * D : (mi + 1) * D], rhs=aT_sb,
                        start=True, stop=True,
                    )
                nc.scalar.activation(
                    e2_sb[:, half * 1024 : (half + 1) * 1024], ps, EXP
                )

            # previous head's output stage slots into the PE's wait for the
            # e2 exps (software pipelining by one head); by the time it's
            # done the first work tile is free again for s1.
            if carry is not None:
                emit_out(h - 1, *carry)

            # s1T[j, n] = sum_i a[i, j] qT[i, n] — BEFORE agg so the e1 exps
            # follow the e2 exps back-to-back on the scalar engine (the two
            # exp chains per head set the pipeline period).
            e1_sb = p_e1.tile([128, N], bf16, tag="e1")
            for half in range(2):
                ps = ps_work.tile([128, 1024], f32, tag="work")
                for t in range(2):
                    c = half * 2 + t
                    nc.tensor.matmul(
                        ps[:, t * 512 : (t + 1) * 512],
                        lhsT=a_sb, rhs=qT_sb[:, c * 512 : (c + 1) * 512],
                        start=True, stop=True,
                    )
                nc.scalar.activation(
                    e1_sb[:, half * 1024 : (half + 1) * 1024], ps, EXP
                )

            # agg[j, 0:128] = sum_m e2[m, j] v[m, :];  agg[j, 128] = S_j
            # (via the ones column baked into the v blob on the host)
            agg = ps_aggp.tile([128, D + 1], f32, tag="agg")
            for mi in range(NCH):
                nc.tensor.matmul(
                    agg,
                    lhsT=e2_sb[:, mi * D : (mi + 1) * D],
                    rhs=v_bf[:, mi * (D + 1) : (mi + 1) * (D + 1)],
                    start=(mi == 0), stop=(mi == NCH - 1),
                )
            recipS = p_sm.tile([128, 1], f32, tag="recipS")
            nc.vector.reciprocal(recipS, agg[:, D : D + 1])
            # aggN has a trailing ones column: the output matmul then yields
            # T_n (the qq softmax denominator) in its own column 128.
            aggN = p_sm.tile([128, D + 1], bf16, tag="aggN")
            nc.gpsimd.memset(aggN[:, D : D + 1], 1.0)
            nc.vector.tensor_tensor(
                aggN[:, 0:D], agg[:, 0:D], recipS.to_broadcast((128, D)), MUL
            )

            if h + PF < H:
                pending.append(load_head(h + PF))
            pending.pop(0)

            carry = (e1_sb, aggN)

        emit_out(H - 1, *carry)

    _split_sync_waits(nc)
    return nc


def _get_program(cfg_key):
    if cfg_key not in _PROGRAM_CACHE:
        _PROGRAM_CACHE[cfg_key] = build_program()
    return _PROGRAM_CACHE[cfg_key]


def kernel(q, a, k, v):
    from concourse.bass_utils import run_bass_kernel_spmd

    import ml_dtypes

    # Device I/O is 16-bit and pre-laid-out: the kernel contracts logits in
    # fp16 and values in bf16 anyway, so rounding + transposing on the host
    # halves HBM traffic and removes all on-chip transposes at no extra
    # precision cost. The fp16 output is upcast back to fp32 below.
    assert np.asarray(q).shape == (B, H, N, D)
    # qT's free dim uses the same row permutation as the output store
    # (position c*128 + p holds query row n = p*16 + c)
    qt = (
        np.asarray(q, dtype=np.float16)
        .reshape(B, H, 128, NCH, D)
        .transpose(0, 1, 4, 3, 2)
        .reshape(B, H, D, N)
    )
    # k chunk-permuted: kp[i, mi*128 + ml] = k[i, 16*ml + mi] so each s2
    # stationary operand is a contiguous 128-col slice
    kp = (
        np.asarray(k, dtype=np.float16)
        .reshape(B, H, D, D, NCH)
        .transpose(0, 1, 2, 4, 3)
        .reshape(B, H, D, N)
    )
    qk = np.empty((B, H, 128, 2 * N), dtype=np.float16)
    qk[..., 0:N] = qt
    qk[..., N : 2 * N] = kp
    qk = np.ascontiguousarray(qk)

    a16 = np.asarray(a, dtype=np.float16)
    # v: rows permuted to n = p*16 + c, a 1.0 column after every 128 values
    # (feeds the kk softmax denominator out of the agg matmul)
    vab = np.empty((B, H, 128, VAB_W), dtype=np.uint16)
    vp = np.ones((B, H, 128, NCH, D + 1), dtype=ml_dtypes.bfloat16)
    vp[..., 0:D] = np.asarray(v, dtype=ml_dtypes.bfloat16).reshape(
        B, H, 128, NCH, D
    )
    vab[..., 0:V_ELEMS] = vp.view(np.uint16).reshape(B, H, 128, V_ELEMS)
    # a_sb[i, j] = a[i, j] (s1 stationary); aT_sb[i, j] = a[j, i] (s2 moving)
    vab[..., A_OFF : A_OFF + D] = a16.view(np.uint16)
    vab[..., AT_OFF : AT_OFF + D] = np.ascontiguousarray(
        a16.transpose(0, 1, 3, 2)
    ).view(np.uint16)
    vab = np.ascontiguousarray(vab).view(np.float16)

    nc = _get_program(("main",))
    core_ids = list(range(NCORES))
    in_maps = [{"qk": qk[c], "vab": vab[c]} for c in core_ids]
    res = run_bass_kernel_spmd(nc, in_maps, core_ids, trace=CONFIG["trace"])
    # o is [H, 128, 16, 128] with (p, c) flattening = natural row n = p*16+c
    out = np.stack(
        [
            np.asarray(res.results[c]["o"], dtype=np.float32).reshape(H, N, D)
            for c in core_ids
        ]
    )
    kernel.last_result = res
    return out


# revision 13
# speedup vs baseline: 1.1311x; 1.0136x over previous
# kernel.py — AgentAttention on 8 Trainium2 NeuronCores (self-contained).
#
# Problem (per batch b, head h):
#   qq  = softmax(q @ a, axis=-1)            # [N, d] over agents d
#   kk  = softmax(a @ k, axis=-1)            # [d, N] over keys N
#   out = qq @ (kk @ v)                      # [N, d]
# Shapes: q [8,16,2048,128], a [8,16,128,128], k [8,16,128,2048],
#         v [8,16,2048,128]; d == n_agents == 128.
#
# Sharding: batch dimension (8) across the 8 cores; each core computes its
# 16 heads independently (pure data parallel, no collectives).
#
# The kernel is HBM-bound (~34MB of fp16/bf16 I/O per core at ~358 GB/s),
# so the structure is built around keeping the DMA rings saturated:
#   - inputs are packed into TWO per-head DRAM blobs so each head costs two
#     big contiguous loads (8KB resp. 4.6KB per partition line): qk blob on
#     the sync HWDGE ring, v+a+aT blob on the scalar HWDGE ring, prefetched
#     two heads ahead (nothing on those rings can ever stall).
#   - the whole head's output is accumulated in SBUF and stored with ONE
#     512KB DMA (4KB lines) per head on the gpsimd SWDGE ring, so the
#     store's data-ready wait can never block load descriptor generation.
# Per-head device algorithm (all matmuls contract over the partition dim):
#   s2T[m,j] = (a@k)^T via lhsT=kp-chunk (host-prepermuted, contiguous),
#              rhs=aT                                              (fp16)
#   e2   = exp(s2T) -> bf16    (no max subtraction: |logit| <~ 70 < 88.7)
#   agg|S = sum_m e2[m,:]^T @ [v_m | 1]  (bf16 matmuls into fp32 psum);
#           col 128 is S_j = sum_m exp, i.e. the kk softmax denominator
#   aggN = agg / S_j                           (row scale, bf16)
#   s1T[j,n] = (q@a)^T via lhsT=a, rhs=qT (pre-transposed)         (fp16)
#   e1   = exp(s1T) -> bf16
#   out  = (e1-chunk^T @ [aggN | 1]) / T_n; the aggN ones column makes
#          col 128 of each psum chunk T_n (the qq softmax denominator)
# fp16 for the logit matmuls: 11-bit mantissa (tf32-class accuracy,
# ~5e-3 abs-max rel err vs 2e-2 budget). The fp16 output is upcast to
# fp32 on the host; q/k/v row permutations are also host-side.

import numpy as np

B, H, N, D = 8, 16, 2048, 128
NCORES = 8
NCH = N // D  # 16 chunks of 128 along the sequence dim

CONFIG = {
    "trace": False,
}

_PROGRAM_CACHE = {}

# per-partition fp16-element offsets into the vab blob
V_ELEMS = NCH * (D + 1)          # 2064 bf16 values (v rows + ones cols)
A_OFF = V_ELEMS                  # a at [2064, 2192)
AT_OFF = V_ELEMS + D             # aT at [2192, 2320)
VAB_W = V_ELEMS + 2 * D          # 2320 fp16 elems = 4640B per partition


def _patch_tile_drain():
    """This container's walrus rejects >1 sync-wait on a Drain instruction
    (CoreV3GenImpl setupSyncWait). Split the TileContext tail-drain's waits
    across consecutive single-wait drains on the same engine; semantics are
    identical (program order ANDs the waits)."""
    import concourse.tile as tile_mod
    from concourse import mybir
    from concourse.tile import ScopedClock

    if getattr(tile_mod.TileContext, "_agentattn_drain_patched", False):
        return

    def _drain_and_barrier(self, tick_clock, wait_clock):
        nc = self.nc
        drain_inst = nc.sync.drain()
        wait_clock.add_sem_waits(
            drain_inst.ins, ScopedClock({None: tick_clock.global_clock})
        )
        si = drain_inst.ins.sync_info
        if si is not None and si.on_wait and len(si.on_wait) > 1:
            waits = list(si.on_wait)
            ups = list(si.on_update or [])
            drain_inst.ins.sync_info = mybir.SyncInfo(
                on_wait=waits[:1], on_update=ups
            )
            for w in waits[1:]:
                d2 = nc.sync.drain()
                d2.ins.sync_info = mybir.SyncInfo(on_wait=[w], on_update=[])
        nc.all_engine_barrier()
        assert self.sems is not None
        popped = nc._tile_sem_poison_stack.pop()
        assert popped is self._sem_poison
        nc.clear_and_free_semaphores(list(self.sems.allocated().values()))
        nc.all_engine_barrier()

    tile_mod.TileContext._drain_and_barrier = _drain_and_barrier
    tile_mod.TileContext._agentattn_drain_patched = True


def _split_sync_waits(nc, max_waits=1):
    """This container's walrus rejects instructions carrying more than one
    sync-wait command. Hoist excess waits onto same-engine NOPs inserted
    immediately before the instruction (program order on the engine ANDs
    the waits, so semantics are unchanged)."""
    from concourse import mybir

    n_split = 0
    for fn in nc.m.functions:
        for blk in fn.blocks:
            insts = blk.instructions
            if not any(
                (si := inst.sync_info) is not None
                and si.on_wait
                and len(si.on_wait) > max_waits
                for inst in insts
            ):
                continue
            new = []
            for inst in insts:
                si = inst.sync_info
                if si is not None and si.on_wait and len(si.on_wait) > max_waits:
                    waits = list(si.on_wait)
                    for idx, w in enumerate(waits[:-max_waits]):
                        nop = mybir.InstNoOp(
                            name=f"{inst.name}_hw{idx}", ins=[], outs=[]
                        )
                        nop.engine = inst.engine
                        nop.sync_info = mybir.SyncInfo(on_wait=[w], on_update=[])
                        nc.register_instruction(nop)
                        new.append(nop)
                        n_split += 1
                    inst.sync_info = mybir.SyncInfo(
                        on_wait=waits[-max_waits:],
                        on_update=list(si.on_update or []),
                    )
                new.append(inst)
            blk.instructions = new
    return n_split


def install_ntff_hook():
    """Make trace=True work in this container: provide the antenv.axon_hooks
    shim that run_bass_kernel_spmd expects, backed by the injected
    libaxon_pjrt.so, and stub out the artifact upload."""
    import sys, types
    if "antenv.axon_hooks" not in sys.modules:
        from trn_agent_boot.trn_boot import _ntff_profile_via_ctypes
        hook = _ntff_profile_via_ctypes("/opt/axon/libaxon_pjrt.so")
        mod = types.ModuleType("antenv.axon_hooks")
        mod.get_axon_ntff_profile_hook = lambda: hook
        mod.set_axon_ntff_profile_hook = lambda h: None
        sys.modules["antenv.axon_hooks"] = mod
    import concourse.bass_utils as bu
    bu.upload_artifacts = lambda tmpdir: tmpdir


def build_program(cfg=None):
    """Build the single-core Bass program (16 heads of agent attention).

    Host-baked DRAM layout (see kernel()):
      qk  [H, 128, 4096] fp16: per partition i: qT[i, 0:2048] | kp[i, 0:2048]
          where qT[d, n'] = q[n, d] with n-position n' = c*128 + p for query
          row n = p*16 + c (the output-store row permutation), and
          kp[i, mi*128 + ml] = k[i, 16*ml + mi] (chunk-contiguous so the s2
          stationary operand loads from a contiguous slice).
      vab [H, 128, 2320] fp16-sized: per partition p:
          v-perm bf16 [16 chunks x 129] (row n = p*16 + c, a 1.0 column
          after every 128 values: feeds the kk softmax denominator) |
          a fp16 [128] | aT fp16 [128]
      o   [H, 128, 16, 128] fp16: partition p, chunk c = output row p*16+c
          (flattening (p,c) IS the natural row order).
    """
    import concourse.bass as bass
    import concourse.tile as tile
    from concourse import mybir
    from contextlib import ExitStack

    _patch_tile_drain()

    f32 = mybir.dt.float32
    f16 = mybir.dt.float16
    bf16 = mybir.dt.bfloat16
    EXP = mybir.ActivationFunctionType.Exp
    MUL = mybir.AluOpType.mult

    nc = bass.Bass("TRN2", target_bir_lowering=False, debug=False)
    qk_d = nc.dram_tensor("qk", [H, 128, 2 * N], f16, kind="ExternalInput").ap()
    vab_d = nc.dram_tensor("vab", [H, 128, VAB_W], f16, kind="ExternalInput").ap()
    # output leaves UNNORMALIZED as bf16 [16 chunks x (128 values | T_n)];
    # the host does the T_n division in fp32 (costs ~2^-9 relative, well
    # inside budget, and removes all reciprocal/multiply work from DVE)
    o_d = nc.dram_tensor(
        "o", [H, 128, NCH * (D + 1)], bf16, kind="ExternalOutput"
    ).ap()

    with tile.TileContext(nc) as tc, ExitStack() as ctx:
        p_qk = ctx.enter_context(tc.tile_pool(name="p_qk", bufs=6))
        p_vab = ctx.enter_context(tc.tile_pool(name="p_vab", bufs=6))
        p_e2 = ctx.enter_context(tc.tile_pool(name="p_e2", bufs=3))
        p_e1 = ctx.enter_context(tc.tile_pool(name="p_e1", bufs=3))
        p_o = ctx.enter_context(tc.tile_pool(name="p_o", bufs=3))
        p_sm = ctx.enter_context(tc.tile_pool(name="p_sm", bufs=3))

        # 8 psum banks exactly: 2x 2-bank work tiles + 1 agg + 3 out
        ps_work = ctx.enter_context(tc.tile_pool(name="ps_work", bufs=2, space="PSUM"))
        ps_aggp = ctx.enter_context(tc.tile_pool(name="ps_agg", bufs=1, space="PSUM"))
        ps_out = ctx.enter_context(tc.tile_pool(name="ps_out", bufs=3, space="PSUM"))

        def load_head(h):
            # two big contiguous loads per head on separate HWDGE rings;
            # nothing else runs on those rings ahead of them, so descriptor
            # generation can never be blocked by a data-dependent wait.
            qk_sb = p_qk.tile([128, 2 * N], f16, tag="qk")
            nc.sync.dma_start(qk_sb, qk_d[h])
            vab_sb = p_vab.tile([128, VAB_W], f16, tag="vab")
            nc.scalar.dma_start(vab_sb, vab_d[h])
            return qk_sb, vab_sb

        PF = 4  # prefetch depth in heads (bufs=6 keeps slot waits clear)
        pending = [load_head(h) for h in range(PF)]

        def emit_out(h, e1_sb, aggN):
            # out'[n, c*129+v] = sum_j e1[j, n] aggN[j, v]; the ones column
            # of aggN makes position 128 of each chunk T_n (the qq softmax
            # denominator). Numerators go to bf16 psum (1024 bf16 = 1 bank,
            # so 7 chunks of 129 fit per bank), DVE just copies them to
            # SBUF, and the host divides. One 528KB store per head.
            o_sb = p_o.tile([128, NCH * (D + 1)], bf16, tag="o_sb")
            GRP = [(0, 3), (3, 3), (6, 3), (9, 3), (12, 3), (15, 1)]
            for g0, gn in GRP:
                pso = ps_out.tile([128, 512], f32, tag="out")
                for i in range(gn):
                    ni = g0 + i
                    nc.tensor.matmul(
                        pso[:, i * (D + 1) : (i + 1) * (D + 1)],
                        lhsT=e1_sb[:, ni * D : (ni + 1) * D], rhs=aggN,
                        start=True, stop=True,
                    )
                nc.vector.tensor_scalar_mul(
                    o_sb[:, g0 * (D + 1) : (g0 + gn) * (D + 1)],
                    pso[:, 0 : gn * (D + 1)],
                    1.0,
                )
            # single whole-head store on the SWDGE ring: its data-ready wait
            # only ever blocks the (idle) gpsimd engine.
            nc.gpsimd.dma_start(o_d[h], o_sb)

        carry = None  # (e1_sb, aggN) of the previous head
        for h in range(H):
            qk_sb, vab_sb = pending[0]
            qT_sb = qk_sb[:, 0:N]
            kp_sb = qk_sb[:, N : 2 * N]
            a_sb = vab_sb[:, A_OFF : A_OFF + D]
            aT_sb = vab_sb[:, AT_OFF : AT_OFF + D]
            v_bf = vab_sb[:, 0:V_ELEMS].bitcast(bf16)

            # s2T[ml, j] for key m = 16*ml + mi; two 2-bank psum halves, one
            # 1024-wide exp each (amortizes the ~270ns ACT fixed cost).
            e2_sb = p_e2.tile([128, N], bf16, tag="e2")
            for half in range(2):
                ps = ps_work.tile([128, 1024], f32, tag="work")
                for t in range(8):
                    mi = half * 8 + t
                    nc.tensor.matmul(
                        ps[:, t * D : (t + 1) * D],
                        lhsT=kp_sb[:, mi * D : (mi + 1) * D], rhs=aT_sb,
                        start=True, stop=True,
                    )
                nc.scalar.activation(
                    e2_sb[:, half * 1024 : (half + 1) * 1024], ps, EXP
                )

            # previous head's output stage slots into the PE's wait for the
            # e2 exps (software pipelining by one head); by the time it's
            # done the first work tile is free again for s1.
            if carry is not None:
                emit_out(h - 1, *carry)

            # s1T[j, n] = sum_i a[i, j] qT[i, n] — BEFORE agg so the e1 exps
            # follow the e2 exps back-to-back on the scalar engine (the two
            # exp chains per head set the pipeline period).
            e1_sb = p_e1.tile([128, N], bf16, tag="e1")
            for half in range(2):
                ps = ps_work.tile([128, 1024], f32, tag="work")
                for t in range(2):
                    c = half * 2 + t
                    nc.tensor.matmul(
                        ps[:, t * 512 : (t + 1) * 512],
                        lhsT=a_sb, rhs=qT_sb[:, c * 512 : (c + 1) * 512],
                        start=True, stop=True,
                    )
                nc.scalar.activation(
                    e1_sb[:, half * 1024 : (half + 1) * 1024], ps, EXP
                )

            # agg[j, 0:128] = sum_m e2[m, j] v[m, :];  agg[j, 128] = S_j
            # (via the ones column baked into the v blob on the host)
            agg = ps_aggp.tile([128, D + 1], f32, tag="agg")
            for mi in range(NCH):
                nc.tensor.matmul(
                    agg,
                    lhsT=e2_sb[:, mi * D : (mi + 1) * D],
                    rhs=v_bf[:, mi * (D + 1) : (mi + 1) * (D + 1)],
                    start=(mi == 0), stop=(mi == NCH - 1),
                )
            recipS = p_sm.tile([128, 1], f32, tag="recipS")
            nc.vector.reciprocal(recipS, agg[:, D : D + 1])
            # aggN has a trailing ones column: the output matmul then yields
            # T_n (the qq softmax denominator) in its own column 128.
            aggN = p_sm.tile([128, D + 1], bf16, tag="aggN")
            nc.gpsimd.memset(aggN[:, D : D + 1], 1.0)
            nc.vector.tensor_tensor(
                aggN[:, 0:D], agg[:, 0:D], recipS.to_broadcast((128, D)), MUL
            )

            if h + PF < H:
                pending.append(load_head(h + PF))
            pending.pop(0)

            carry = (e1_sb, aggN)

        emit_out(H - 1, *carry)

    _split_sync_waits(nc)
    return nc


def _get_program(cfg_key):
    if cfg_key not in _PROGRAM_CACHE:
        _PROGRAM_CACHE[cfg_key] = build_program()
    return _PROGRAM_CACHE[cfg_key]


def kernel(q, a, k, v):
    from concourse.bass_utils import run_bass_kernel_spmd

    import ml_dtypes

    # Device I/O is 16-bit and pre-laid-out: the kernel contracts logits in
    # fp16 and values in bf16 anyway, so rounding + transposing on the host
    # halves HBM traffic and removes all on-chip transposes at no extra
    # precision cost. The fp16 output is upcast back to fp32 below.
    assert np.asarray(q).shape == (B, H, N, D)
    # qT's free dim uses the same row permutation as the output store
    # (position c*128 + p holds query row n = p*16 + c)
    qt = (
        np.asarray(q, dtype=np.float16)
        .reshape(B, H, 128, NCH, D)
        .transpose(0, 1, 4, 3, 2)
        .reshape(B, H, D, N)
    )
    # k chunk-permuted: kp[i, mi*128 + ml] = k[i, 16*ml + mi] so each s2
    # stationary operand is a contiguous 128-col slice
    kp = (
        np.asarray(k, dtype=np.float16)
        .reshape(B, H, D, D, NCH)
        .transpose(0, 1, 2, 4, 3)
        .reshape(B, H, D, N)
    )
    qk = np.empty((B, H, 128, 2 * N), dtype=np.float16)
    qk[..., 0:N] = qt
    qk[..., N : 2 * N] = kp
    qk = np.ascontiguousarray(qk)

    a16 = np.asarray(a, dtype=np.float16)
    # v: rows permuted to n = p*16 + c, a 1.0 column after every 128 values
    # (feeds the kk softmax denominator out of the agg matmul)
    vab = np.empty((B, H, 128, VAB_W), dtype=np.uint16)
    vp = np.ones((B, H, 128, NCH, D + 1), dtype=ml_dtypes.bfloat16)
    vp[..., 0:D] = np.asarray(v, dtype=ml_dtypes.bfloat16).reshape(
        B, H, 128, NCH, D
    )
    vab[..., 0:V_ELEMS] = vp.view(np.uint16).reshape(B, H, 128, V_ELEMS)
    # a_sb[i, j] = a[i, j] (s1 stationary); aT_sb[i, j] = a[j, i] (s2 moving)
    vab[..., A_OFF : A_OFF + D] = a16.view(np.uint16)
    vab[..., AT_OFF : AT_OFF + D] = np.ascontiguousarray(
        a16.transpose(0, 1, 3, 2)
    ).view(np.uint16)
    vab = np.ascontiguousarray(vab).view(np.float16)

    nc = _get_program(("main",))
    core_ids = list(range(NCORES))
    in_maps = [{"qk": qk[c], "vab": vab[c]} for c in core_ids]
    res = run_bass_kernel_spmd(nc, in_maps, core_ids, trace=CONFIG["trace"])
    # o is [H, 128, 16*(128|T)] unnormalized bf16; divide by the T_n column
    # in fp32 here, with (p, c) flattening = natural row n = p*16+c
    outs = []
    for c in core_ids:
        o = np.asarray(res.results[c]["o"], dtype=np.float32).reshape(
            H, 128, NCH, D + 1
        )
        outs.append((o[..., 0:D] / o[..., D : D + 1]).reshape(H, N, D))
    out = np.stack(outs)
    kernel.last_result = res
    return out
